# revision 13
# baseline (speedup 1.0000x reference)
"""Autoregressive LSTM (B=64, H=1024, T=256) + LayerNorm + projection (D=512)
on 8 Trainium2 NeuronCores.

Parallelization: tensor-parallel over the hidden dim for the sequential
recurrence (core c owns h-indices [128c, 128c+128)), with a per-step AllGather
of the transposed hidden state; then a batch-sliced LayerNorm + projection.

Key identity: after step 0 the LSTM input equals its own hidden state, so
gates = h @ (W_ih + W_hh).T + (b_ih + b_hh) for t >= 1, and
gates = x @ W_ih.T + (b_ih + b_hh) for t = 0 (h = c = 0).

LayerNorm is folded into the projection:
out = rstd*(h @ W1.T) + (-rstd*mu) x A + 1 x C,  W1 = gamma*W_out,
A[d] = sum_j W1[d,j], C = beta @ W_out.T + b_out.
"""
import os
import sys
sys.path.insert(0, "/opt/trn_rl_repo")
import numpy as np

import concourse.bass as bass
import concourse.bacc as bacc
from concourse import tile, mybir
from concourse.bass_utils import run_bass_kernel_spmd

F32 = mybir.dt.float32
AF = mybir.ActivationFunctionType

NC = 8          # cores
B = 64          # batch
H = 1024        # hidden
D = 512         # output dim
T_FULL = 256    # sequence length
HS = H // NC    # per-core hidden slice (128)
BL = B // NC    # per-core batch slice for phase 2 (8)
LN_EPS = 1e-5

_CACHE = {}


def _build(t_steps: int, comm: str = 'ag', prec: str = 'f32r'):
    if prec == 'f16':
        DT = mybir.dt.float16
    elif prec == 'f32r':
        DT = mybir.dt.float32r
    else:
        DT = F32
    nc = bacc.Bacc(None, num_devices=NC)

    # ---- per-core external inputs ----
    w0T = nc.dram_tensor("w0T", [128, NC * 512], DT, kind="ExternalInput")
    wT = nc.dram_tensor("wT", [128, NC * 512], DT, kind="ExternalInput")
    biasT = nc.dram_tensor("biasT", [1, 512], DT, kind="ExternalInput")
    xT = nc.dram_tensor("xT", [128, NC * B], DT, kind="ExternalInput")
    identB = nc.dram_tensor("identB", [B, B], F32, kind="ExternalInput")
    w1T = nc.dram_tensor("w1T", [128, NC * D], DT, kind="ExternalInput")
    avec = nc.dram_tensor("avec", [1, D], DT, kind="ExternalInput")
    cvec = nc.dram_tensor("cvec", [1, D], DT, kind="ExternalInput")
    ones128_in = nc.dram_tensor("ones128", [128, 1], DT, kind="ExternalInput")
    onesB_in = nc.dram_tensor("onesB", [1, B], DT, kind="ExternalInput")
    onesrow_in = nc.dram_tensor("ones_row", [1, 128], DT, kind="ExternalInput")

    out = nc.dram_tensor("out", [T_FULL * BL, D], F32, kind="ExternalOutput")

    cc_in = nc.dram_tensor("cc_in", [HS, B], DT)
    cc_out = nc.dram_tensor("cc_out", [H, B], DT, addr_space="Shared")

    n_tg = (t_steps + 63) // 64       # 64-step stat groups
    n_mg_per_tg = 4                   # 16-step matmul groups per stat group

    with tile.TileContext(nc) as tc:
        with (
            tc.tile_pool(name="const", bufs=1) as cp,
            tc.tile_pool(name="seq", bufs=1) as qp,
        ):
            # ---- constants ----
            w0T_sb = cp.tile([128, NC * 512], DT, tag="w0T")
            nc.sync.dma_start(out=w0T_sb[:, :], in_=w0T[:, :])
            wT_sb = cp.tile([128, NC * 512], DT, tag="wT")
            nc.sync.dma_start(out=wT_sb[:, :], in_=wT[:, :])
            b_sb = cp.tile([1, 512], DT, tag="biasT")
            nc.sync.dma_start(out=b_sb[:, :], in_=biasT[:, :])
            xT_sb = cp.tile([128, NC, B], DT, tag="xT")
            nc.sync.dma_start(out=xT_sb[:, :, :],
                              in_=xT.ap().rearrange("p (k b) -> p k b", k=NC))
            ident = cp.tile([B, B], F32, tag="identB")
            nc.sync.dma_start(out=ident[:, :], in_=identB[:, :])
            w1T_sb = cp.tile([128, NC * D], DT, tag="w1T")
            nc.sync.dma_start(out=w1T_sb[:, :], in_=w1T[:, :])
            a_sb = cp.tile([1, D], DT, tag="avec")
            nc.sync.dma_start(out=a_sb[:, :], in_=avec[:, :])
            c_sb = cp.tile([1, D], DT, tag="cvec")
            nc.sync.dma_start(out=c_sb[:, :], in_=cvec[:, :])
            ones128 = cp.tile([128, 1], DT, tag="ones128")
            nc.sync.dma_start(out=ones128[:, :], in_=ones128_in[:, :])
            ones_row = cp.tile([1, 128], DT, tag="ones_row")
            nc.sync.dma_start(out=ones_row[:, :], in_=onesrow_in[:, :])
            onesB = cp.tile([1, B], DT, tag="onesB")
            nc.sync.dma_start(out=onesB[:, :], in_=onesB_in[:, :])
            eps_t = cp.tile([1, 1], F32, tag="eps")
            nc.vector.memset(eps_t[:, :], LN_EPS)

            # seqT[p, k, t, b] = h_t[8*pid + b, 128k + p]
            seqT = qp.tile([128, NC, t_steps, BL], DT, tag="seqT")

            pid = nc.vector.partition_id()
            bsel = bass.ts(pid, BL)  # dynamic slice [8*pid, 8*pid+8)

            # ================= recurrence =================
            with (
                tc.tile_pool(name="state", bufs=2) as sp,
                tc.tile_pool(name="ps", bufs=2, space="PSUM") as pp,
            ):
                if comm == 'rd':
                    rsem = nc.alloc_semaphore("rdma_recv")
                    lsem = nc.alloc_semaphore("rdma_sent")
                deferred_waits = []

                n_warm = int(os.environ.get("KWARM", "32"))
                if comm == 'ag2':
                    warm_ps = pp.tile([B, 512], F32, tag="warm", bufs=1)

                c_prev = None
                hT_all_prev = None
                for t in range(t_steps):
                    w_cur = w0T_sb if t == 0 else wT_sb
                    src_all = xT_sb if t == 0 else hT_all_prev

                    # gates[b, fiog] = bias + sum_k h_k.T @ w_k
                    g_ps = pp.tile([B, 512], F32, tag="g")
                    nc.tensor.matmul(g_ps[:, :], onesB[:, :], b_sb[:, :],
                                     start=True, stop=False)
                    for k in range(NC):
                        mm = nc.tensor.matmul(
                            g_ps[:, :],
                            src_all[:, k, :],
                            w_cur[:, k * 512:(k + 1) * 512],
                            start=False, stop=(k == NC - 1),
                        )
                        if comm == 'rd' and t >= 1 and k == 0:
                            # gate on arrival of all 7 peer chunks of step t-1
                            deferred_waits.append((mm, rsem, 14 * t))

                    # eltwise, split so DVE overlaps ACT:
                    # sigmoid(f,i) -> tanh(g) -> sigmoid(o) on ACT while the
                    # DVE cell-state chain runs between them.
                    s_fi = sp.tile([B, 256], F32, tag="sfi")
                    nc.scalar.activation(s_fi[:, :], g_ps[:, 0:256], AF.Sigmoid)
                    tg = sp.tile([B, 128], F32, tag="tg")
                    nc.scalar.activation(tg[:, :], g_ps[:, 384:512], AF.Tanh)
                    s_o = sp.tile([B, 128], F32, tag="so")
                    nc.scalar.activation(s_o[:, :], g_ps[:, 256:384], AF.Sigmoid)

                    c_new = sp.tile([B, 128], F32, tag="c")
                    if t == 0:
                        nc.vector.tensor_mul(c_new[:, :], s_fi[:, 128:256], tg[:, :])
                    else:
                        t1 = sp.tile([B, 128], F32, tag="t1")
                        nc.vector.tensor_mul(t1[:, :], s_fi[:, 0:128], c_prev[:, :])
                        t2 = sp.tile([B, 128], F32, tag="t2")
                        nc.vector.tensor_mul(t2[:, :], s_fi[:, 128:256], tg[:, :])
                        nc.vector.tensor_add(c_new[:, :], t1[:, :], t2[:, :])
                    c_prev = c_new

                    tc_t = sp.tile([B, 128], F32, tag="tc")
                    nc.scalar.activation(tc_t[:, :], c_new[:, :], AF.Tanh)
                    h_new = sp.tile([B, 128], F32, tag="h")
                    nc.vector.tensor_mul(h_new[:, :], s_o[:, :], tc_t[:, :])

                    # transpose h [64,128] -> [128,64], copy to SBUF
                    hT_ps = pp.tile([HS, B], F32, tag="hT")
                    nc.tensor.transpose(hT_ps[:, :], h_new[:, :], ident[:, :])

                    if comm == 'rd':
                        # own chunk -> slot 0 of the parity buffer; peers fill
                        # slots 1..7 (slot j on core c <- chunk of core c^j).
                        hT_all = sp.tile([128, NC, B], DT, tag="hT_all")
                        cp_i = nc.scalar.copy(out=hT_all[:, 0, :], in_=hT_ps[:, :])
                        if t >= 2:
                            # slot reuse: sends of step t-2 from this buffer done
                            deferred_waits.append((cp_i, lsem, 112 * (t - 1)))
                        for j in range(1, NC):
                            rdests = [None] * NC
                            rdests[j] = (0, j)
                            nc.gpsimd.remote_dma_broadcast(
                                out_ap=hT_all[:, j, :],
                                in_ap=hT_all[:, 0, :],
                                remote_sem=rsem,
                                local_sem=lsem,
                                rdests=rdests,
                            )
                        nc.gpsimd.trigger_dma(count=None)
                        hT_all_prev = hT_all
                        sc = nc.vector.tensor_copy(seqT[:, :, t, :],
                                                   hT_all[:, :, bsel])
                        deferred_waits.append((sc, rsem, 14 * (t + 1)))
                    elif comm == 'ag2':
                        # HWDGE staging + PE-warming dummies during the AG wait
                        hT = sp.tile([HS, B], DT, tag="hTsb")
                        nc.scalar.copy(out=hT[:, :], in_=hT_ps[:, :])
                        nc.sync.dma_start(out=cc_in[:, :], in_=hT[:, :])
                        nc.gpsimd.collective_compute(
                            "AllGather",
                            mybir.AluOpType.bypass,
                            replica_groups=[list(range(NC))],
                            ins=[cc_in.ap().opt()],
                            outs=[cc_out.ap().opt()],
                        )
                        # dummy MMs anchored on this step's hT (so Tile can't
                        # hoist them) and with no AG dependency: they execute
                        # during the collective wait and keep HAM at 8/8
                        for w in range(n_warm):
                            nc.tensor.matmul(warm_ps[:, :], hT[:, :],
                                             wT_sb[:, (w % 8) * 512:(w % 8) * 512 + 512],
                                             start=True, stop=True)
                        hT_all = sp.tile([128, NC, B], DT, tag="hT_all")
                        nc.sync.dma_start(
                            out=hT_all[:, :, :],
                            in_=cc_out.ap().rearrange("(k p) b -> p k b", k=NC),
                        )
                        hT_all_prev = hT_all
                        nc.vector.tensor_copy(seqT[:, :, t, :], hT_all[:, :, bsel])
                    else:
                        hT = sp.tile([HS, B], DT, tag="hTsb")
                        nc.scalar.copy(out=hT[:, :], in_=hT_ps[:, :])
                        nc.gpsimd.dma_start(out=cc_in[:, :], in_=hT[:, :])
                        if comm == 'ag':
                            nc.gpsimd.collective_compute(
                                "AllGather",
                                mybir.AluOpType.bypass,
                                replica_groups=[list(range(NC))],
                                ins=[cc_in.ap().opt()],
                                outs=[cc_out.ap().opt()],
                            )
                        else:
                            nc.gpsimd.dma_start(out=cc_out[0:HS, :], in_=cc_in[:, :])
                        hT_all = sp.tile([128, NC, B], DT, tag="hT_all")
                        nc.gpsimd.dma_start(
                            out=hT_all[:, :, :],
                            in_=cc_out.ap().rearrange("(k p) b -> p k b", k=NC),
                        )
                        hT_all_prev = hT_all
                        # stash my batch slice: seqT[:, t, k, b] = hT_all[:, k, 8p+b]
                        nc.vector.tensor_copy(seqT[:, :, t, :], hT_all[:, :, bsel])

            # ================= phase 2: LN + projection =================
            with (
                tc.tile_pool(name="p2", bufs=2) as p2,
                tc.tile_pool(name="ps2", bufs=2, space="PSUM") as pp2,
            ):
                inv_h = 1.0 / H
                for tg_i in range(n_tg):
                    t0 = tg_i * 64
                    tn = min(64, t_steps - t0)
                    ncols = tn * NC * BL

                    # squares
                    sq = p2.tile([128, NC, 64, BL], DT, tag="sq")
                    nc.scalar.activation(sq[:, :, 0:tn, :],
                                         seqT[:, :, t0:t0 + tn, :], AF.Square)

                    # partition sums via ones-matmul, accumulated over k-chunks
                    mu_ps = pp2.tile([1, 512], F32, tag="mu", bufs=1)
                    sqs_ps = pp2.tile([1, 512], F32, tag="sqs", bufs=1)
                    for k in range(NC):
                        nc.tensor.matmul(
                            mu_ps[:, 0:tn * BL], ones128[:, :],
                            seqT[:, k, t0:t0 + tn, :].opt(),
                            start=(k == 0), stop=(k == NC - 1))
                    for k in range(NC):
                        nc.tensor.matmul(
                            sqs_ps[:, 0:tn * BL], ones128[:, :],
                            sq[:, k, 0:tn, :].opt(),
                            start=(k == 0), stop=(k == NC - 1))

                    # stats: mean, var, rstd, -rstd*mean
                    mean = p2.tile([1, 512], F32, tag="mean")
                    nc.vector.tensor_scalar_mul(mean[:, 0:tn * BL],
                                                mu_ps[:, 0:tn * BL], inv_h)
                    exsq = p2.tile([1, 512], F32, tag="exsq")
                    nc.vector.tensor_scalar_mul(exsq[:, 0:tn * BL],
                                                sqs_ps[:, 0:tn * BL], inv_h)
                    msq = p2.tile([1, 512], F32, tag="msq")
                    nc.vector.tensor_mul(msq[:, 0:tn * BL], mean[:, 0:tn * BL],
                                         mean[:, 0:tn * BL])
                    var = p2.tile([1, 512], F32, tag="var")
                    nc.vector.tensor_sub(var[:, 0:tn * BL], exsq[:, 0:tn * BL],
                                         msq[:, 0:tn * BL])
                    std = p2.tile([1, 512], F32, tag="std")
                    nc.scalar.activation(std[:, 0:tn * BL], var[:, 0:tn * BL],
                                         AF.Sqrt, bias=eps_t[0:1, 0:1])
                    rstd = p2.tile([1, 512], F32, tag="rstd")
                    nc.vector.reciprocal(rstd[:, 0:tn * BL], std[:, 0:tn * BL])
                    negrm = p2.tile([1, 512], DT, tag="negrm")
                    nc.vector.scalar_tensor_tensor(
                        negrm[:, 0:tn * BL], rstd[:, 0:tn * BL], -1.0,
                        mean[:, 0:tn * BL],
                        op0=mybir.AluOpType.mult, op1=mybir.AluOpType.mult)

                    n_mg = (tn + 15) // 16
                    for sub in range(n_mg):
                        t0m = t0 + sub * 16
                        tm = min(16, t_steps - t0m)
                        mrows = tm * BL

                        # rstd as per-partition column
                        rcol_ps = pp2.tile([128, 1], F32, tag="rcol", bufs=1)
                        nc.tensor.transpose(
                            rcol_ps[0:mrows, :],
                            rstd[:, sub * 128:sub * 128 + mrows],
                            ident[0:1, 0:1])
                        rcol = p2.tile([128, 1], F32, tag="rcolsb")
                        nc.scalar.copy(out=rcol[0:mrows, :], in_=rcol_ps[0:mrows, :])

                        # projection
                        p_ps = pp2.tile([128, D], F32, tag="P")
                        for k in range(NC):
                            nc.tensor.matmul(
                                p_ps[0:mrows, :],
                                seqT[:, k, t0m:t0m + tm, :].opt(),
                                w1T_sb[:, k * D:(k + 1) * D],
                                start=(k == 0), stop=(k == NC - 1))
                        # rank-1 terms in a separate accumulator
                        r1_ps = pp2.tile([128, D], F32, tag="r1")
                        nc.tensor.matmul(r1_ps[0:mrows, :],
                                         negrm[:, sub * 128:sub * 128 + mrows],
                                         a_sb[:, :], start=True, stop=False)
                        nc.tensor.matmul(r1_ps[0:mrows, :],
                                         ones_row[:, 0:mrows],
                                         c_sb[:, :], start=False, stop=True)

                        scaled = p2.tile([128, D], F32, tag="scaled")
                        nc.scalar.activation(scaled[0:mrows, :], p_ps[0:mrows, :],
                                             AF.Copy, scale=rcol[0:mrows, :])
                        fin = p2.tile([128, D], F32, tag="fin")
                        nc.vector.tensor_add(fin[0:mrows, :], scaled[0:mrows, :],
                                             r1_ps[0:mrows, :])
                        g16 = tg_i * n_mg_per_tg + sub
                        nc.sync.dma_start(
                            out=out[g16 * 128:g16 * 128 + mrows, :],
                            in_=fin[0:mrows, :])

    # Cross-core gates attached AFTER Tile scheduling: Tile's single-core
    # scheduling sim can't satisfy remotely-incremented semaphores.
    for inst, sem, val in deferred_waits:
        inst.wait_op(sem, val, "sem-ge", check=False)
    nc.finalize()
    return nc


def _prep_inputs(x, W_ih, W_hh, b_ih, b_hh, ln_gamma, ln_beta, W_out, b_out,
                 comm='ag', prec='f32r'):
    """Host-side sharding. Returns per-core input maps.

    For comm='rd' the K-chunk slots are XOR-permuted per core: slot j on
    core c holds chunk c^j (matching the remote-dma broadcast pattern)."""
    npdt = np.float16 if prec == 'f16' else np.float32
    Wc = (W_ih + W_hh).astype(np.float32)
    bb = (b_ih + b_hh).astype(np.float32)

    def chunk_major_T(wslice, perm):
        # [512, 1024] -> [128, 8*512]: slot s <- K-chunk perm[s]
        wt = wslice.T.reshape(NC, 128, wslice.shape[0])[perm]
        return np.ascontiguousarray(wt.transpose(1, 0, 2).reshape(128, -1))

    def x_cm(perm):
        return np.ascontiguousarray(
            x.T.reshape(NC, 128, B)[perm].transpose(1, 0, 2).reshape(128, NC * B)
        ).astype(np.float32)

    W1 = (W_out * ln_gamma[None, :]).astype(np.float32)       # [512, 1024]
    a_v = W1.sum(axis=1).astype(np.float32).reshape(1, D)
    c_v = (ln_beta @ W_out.T + b_out).astype(np.float32).reshape(1, D)
    identB = np.eye(B, dtype=np.float32)
    ones128 = np.ones((128, 1), dtype=np.float32)

    in_maps = []
    for c in range(NC):
        perm = [c ^ j for j in range(NC)] if comm == 'rd' else list(range(NC))

        def rows(W):
            return np.concatenate([
                W[1024 + 128 * c:1024 + 128 * c + 128],   # f
                W[0 + 128 * c:0 + 128 * c + 128],          # i
                W[3072 + 128 * c:3072 + 128 * c + 128],    # o
                W[2048 + 128 * c:2048 + 128 * c + 128],    # g
            ], axis=0)

        bias_fiog = np.concatenate([
            bb[1024 + 128 * c:1024 + 128 * c + 128],
            bb[0 + 128 * c:0 + 128 * c + 128],
            bb[3072 + 128 * c:3072 + 128 * c + 128],
            bb[2048 + 128 * c:2048 + 128 * c + 128],
        ]).reshape(1, 512).astype(np.float32)

        in_maps.append({
            "w0T": chunk_major_T(rows(W_ih.astype(np.float32)), perm).astype(npdt),
            "wT": chunk_major_T(rows(Wc), perm).astype(npdt),
            "biasT": bias_fiog.astype(npdt),
            "xT": x_cm(perm).astype(npdt),
            "identB": identB,
            "w1T": chunk_major_T(W1, perm).astype(npdt),
            "avec": a_v.astype(npdt),
            "cvec": c_v.astype(npdt),
            "ones128": ones128.astype(npdt),
            "onesB": np.ones((1, B), dtype=npdt),
            "ones_row": np.ones((1, 128), dtype=npdt),
        })
    return in_maps


def kernel(x, W_ih, W_hh, b_ih, b_hh, ln_gamma, ln_beta, W_out, b_out,
           _t=T_FULL, _trace=False, _comm='ag2', _prec='f16'):
    t_steps = _t
    key = (t_steps, _comm, _prec)
    if key not in _CACHE:
        _CACHE[key] = _build(t_steps, _comm, _prec)
    nc = _CACHE[key]

    in_maps = _prep_inputs(x, W_ih, W_hh, b_ih, b_hh,
                           ln_gamma, ln_beta, W_out, b_out, comm=_comm,
                           prec=_prec)
    res = run_bass_kernel_spmd(nc, in_maps, core_ids=list(range(NC)),
                               trace=_trace)

    out = np.empty((B, t_steps, D), dtype=np.float32)
    for c in range(NC):
        r = res.results[c]["out"][:t_steps * BL].reshape(t_steps, BL, D)
        out[BL * c:BL * c + BL] = r.transpose(1, 0, 2)
    if _trace:
        kernel.last_result = res
    return out



# revision 14
# speedup vs baseline: 1.0680x; 1.0680x over previous
"""Autoregressive LSTM (B=64, H=1024, T=256) + LayerNorm + projection (D=512)
on 8 Trainium2 NeuronCores.

Parallelization: tensor-parallel over the hidden dim for the sequential
recurrence (core c owns h-indices [128c, 128c+128)), with a per-step AllGather
of the transposed hidden state; then a batch-sliced LayerNorm + projection.

Key identity: after step 0 the LSTM input equals its own hidden state, so
gates = h @ (W_ih + W_hh).T + (b_ih + b_hh) for t >= 1, and
gates = x @ W_ih.T + (b_ih + b_hh) for t = 0 (h = c = 0).

LayerNorm is folded into the projection:
out = rstd*(h @ W1.T) + (-rstd*mu) x A + 1 x C,  W1 = gamma*W_out,
A[d] = sum_j W1[d,j], C = beta @ W_out.T + b_out.
"""
import os
import sys
sys.path.insert(0, "/opt/trn_rl_repo")
import numpy as np

import concourse.bass as bass
import concourse.bacc as bacc
from concourse import tile, mybir
from concourse.bass_utils import run_bass_kernel_spmd

F32 = mybir.dt.float32
AF = mybir.ActivationFunctionType

NC = 8          # cores
B = 64          # batch
H = 1024        # hidden
D = 512         # output dim
T_FULL = 256    # sequence length
HS = H // NC    # per-core hidden slice (128)
BL = B // NC    # per-core batch slice for phase 2 (8)
LN_EPS = 1e-5

_CACHE = {}


def _build(t_steps: int, comm: str = 'ag', prec: str = 'f32r'):
    if prec == 'f16':
        DT = mybir.dt.float16
    elif prec == 'f32r':
        DT = mybir.dt.float32r
    else:
        DT = F32
    nc = bacc.Bacc(None, num_devices=NC)

    # ---- per-core external inputs ----
    w0T = nc.dram_tensor("w0T", [128, NC * 512], DT, kind="ExternalInput")
    wT = nc.dram_tensor("wT", [128, NC * 512], DT, kind="ExternalInput")
    biasT = nc.dram_tensor("biasT", [1, 512], DT, kind="ExternalInput")
    xT = nc.dram_tensor("xT", [128, NC * B], DT, kind="ExternalInput")
    identB = nc.dram_tensor("identB", [B, B], F32, kind="ExternalInput")
    w1T = nc.dram_tensor("w1T", [128, NC * D], DT, kind="ExternalInput")
    avec = nc.dram_tensor("avec", [1, D], DT, kind="ExternalInput")
    cvec = nc.dram_tensor("cvec", [1, D], DT, kind="ExternalInput")
    ones128_in = nc.dram_tensor("ones128", [128, 1], DT, kind="ExternalInput")
    onesB_in = nc.dram_tensor("onesB", [1, B], DT, kind="ExternalInput")
    onesrow_in = nc.dram_tensor("ones_row", [1, 128], DT, kind="ExternalInput")

    out = nc.dram_tensor("out", [T_FULL * BL, D], F32, kind="ExternalOutput")

    cc_in = nc.dram_tensor("cc_in", [HS, B], DT)
    cc_out = nc.dram_tensor("cc_out", [H, B], DT, addr_space="Shared")

    n_tg = (t_steps + 63) // 64       # 64-step stat groups
    n_mg_per_tg = 4                   # 16-step matmul groups per stat group

    with tile.TileContext(nc) as tc:
        with (
            tc.tile_pool(name="const", bufs=1) as cp,
            tc.tile_pool(name="seq", bufs=1) as qp,
        ):
            # ---- constants ----
            w0T_sb = cp.tile([128, NC * 512], DT, tag="w0T")
            nc.sync.dma_start(out=w0T_sb[:, :], in_=w0T[:, :])
            wT_sb = cp.tile([128, NC * 512], DT, tag="wT")
            nc.sync.dma_start(out=wT_sb[:, :], in_=wT[:, :])
            b_sb = cp.tile([1, 512], DT, tag="biasT")
            nc.sync.dma_start(out=b_sb[:, :], in_=biasT[:, :])
            xT_sb = cp.tile([128, NC, B], DT, tag="xT")
            nc.sync.dma_start(out=xT_sb[:, :, :],
                              in_=xT.ap().rearrange("p (k b) -> p k b", k=NC))
            ident = cp.tile([B, B], F32, tag="identB")
            nc.sync.dma_start(out=ident[:, :], in_=identB[:, :])
            w1T_sb = cp.tile([128, NC * D], DT, tag="w1T")
            nc.sync.dma_start(out=w1T_sb[:, :], in_=w1T[:, :])
            a_sb = cp.tile([1, D], DT, tag="avec")
            nc.sync.dma_start(out=a_sb[:, :], in_=avec[:, :])
            c_sb = cp.tile([1, D], DT, tag="cvec")
            nc.sync.dma_start(out=c_sb[:, :], in_=cvec[:, :])
            ones128 = cp.tile([128, 1], DT, tag="ones128")
            nc.sync.dma_start(out=ones128[:, :], in_=ones128_in[:, :])
            ones_row = cp.tile([1, 128], DT, tag="ones_row")
            nc.sync.dma_start(out=ones_row[:, :], in_=onesrow_in[:, :])
            onesB = cp.tile([1, B], DT, tag="onesB")
            nc.sync.dma_start(out=onesB[:, :], in_=onesB_in[:, :])
            eps_t = cp.tile([1, 1], F32, tag="eps")
            nc.vector.memset(eps_t[:, :], LN_EPS)

            # seqT[p, k, t, b] = h_t[8*pid + b, 128k + p]
            seqT = qp.tile([128, NC, t_steps, BL], DT, tag="seqT")

            pid = nc.vector.partition_id()
            bsel = bass.ts(pid, BL)  # dynamic slice [8*pid, 8*pid+8)

            # ================= recurrence =================
            with (
                tc.tile_pool(name="state", bufs=2) as sp,
                tc.tile_pool(name="ps", bufs=2, space="PSUM") as pp,
            ):
                if comm == 'rd':
                    rsem = nc.alloc_semaphore("rdma_recv")
                    lsem = nc.alloc_semaphore("rdma_sent")
                deferred_waits = []

                n_warm = int(os.environ.get("KWARM", "14"))
                if comm == 'ag2':
                    warm_ps = pp.tile([B, 512], F32, tag="warm", bufs=1)

                c_prev = None
                hT_all_prev = None
                for t in range(t_steps):
                    w_cur = w0T_sb if t == 0 else wT_sb
                    src_all = xT_sb if t == 0 else hT_all_prev

                    # gates[b, fiog] = bias + sum_k h_k.T @ w_k
                    g_ps = pp.tile([B, 512], F32, tag="g")
                    nc.tensor.matmul(g_ps[:, :], onesB[:, :], b_sb[:, :],
                                     start=True, stop=False)
                    for k in range(NC):
                        mm = nc.tensor.matmul(
                            g_ps[:, :],
                            src_all[:, k, :],
                            w_cur[:, k * 512:(k + 1) * 512],
                            start=False, stop=(k == NC - 1),
                        )
                        if comm == 'rd' and t >= 1 and k == 0:
                            # gate on arrival of all 7 peer chunks of step t-1
                            deferred_waits.append((mm, rsem, 14 * t))

                    # eltwise, split so DVE overlaps ACT:
                    # sigmoid(f,i) -> tanh(g) -> sigmoid(o) on ACT while the
                    # DVE cell-state chain runs between them.
                    s_fi = sp.tile([B, 256], F32, tag="sfi")
                    nc.scalar.activation(s_fi[:, :], g_ps[:, 0:256], AF.Sigmoid)
                    tg = sp.tile([B, 128], F32, tag="tg")
                    nc.scalar.activation(tg[:, :], g_ps[:, 384:512], AF.Tanh)
                    s_o = sp.tile([B, 128], F32, tag="so")
                    nc.scalar.activation(s_o[:, :], g_ps[:, 256:384], AF.Sigmoid)

                    c_new = sp.tile([B, 128], F32, tag="c")
                    if t == 0:
                        nc.vector.tensor_mul(c_new[:, :], s_fi[:, 128:256], tg[:, :])
                    else:
                        t1 = sp.tile([B, 128], F32, tag="t1")
                        nc.vector.tensor_mul(t1[:, :], s_fi[:, 0:128], c_prev[:, :])
                        t2 = sp.tile([B, 128], F32, tag="t2")
                        nc.vector.tensor_mul(t2[:, :], s_fi[:, 128:256], tg[:, :])
                        nc.vector.tensor_add(c_new[:, :], t1[:, :], t2[:, :])
                    c_prev = c_new

                    tc_t = sp.tile([B, 128], F32, tag="tc")
                    nc.scalar.activation(tc_t[:, :], c_new[:, :], AF.Tanh)
                    h_new = sp.tile([B, 128], F32, tag="h")
                    nc.vector.tensor_mul(h_new[:, :], s_o[:, :], tc_t[:, :])

                    # transpose h [64,128] -> [128,64], copy to SBUF
                    hT_ps = pp.tile([HS, B], F32, tag="hT")
                    nc.tensor.transpose(hT_ps[:, :], h_new[:, :], ident[:, :])

                    if comm == 'rd':
                        # own chunk -> slot 0 of the parity buffer; peers fill
                        # slots 1..7 (slot j on core c <- chunk of core c^j).
                        hT_all = sp.tile([128, NC, B], DT, tag="hT_all")
                        cp_i = nc.scalar.copy(out=hT_all[:, 0, :], in_=hT_ps[:, :])
                        if t >= 2:
                            # slot reuse: sends of step t-2 from this buffer done
                            deferred_waits.append((cp_i, lsem, 112 * (t - 1)))
                        for j in range(1, NC):
                            rdests = [None] * NC
                            rdests[j] = (0, j)
                            nc.gpsimd.remote_dma_broadcast(
                                out_ap=hT_all[:, j, :],
                                in_ap=hT_all[:, 0, :],
                                remote_sem=rsem,
                                local_sem=lsem,
                                rdests=rdests,
                            )
                        nc.gpsimd.trigger_dma(count=None)
                        hT_all_prev = hT_all
                        sc = nc.vector.tensor_copy(seqT[:, :, t, :],
                                                   hT_all[:, :, bsel])
                        deferred_waits.append((sc, rsem, 14 * (t + 1)))
                    elif comm == 'ag2':
                        # HWDGE staging + PE-warming dummies during the AG wait
                        hT = sp.tile([HS, B], DT, tag="hTsb")
                        nc.scalar.copy(out=hT[:, :], in_=hT_ps[:, :])
                        nc.sync.dma_start(out=cc_in[:, :], in_=hT[:, :])
                        nc.gpsimd.collective_compute(
                            "AllGather",
                            mybir.AluOpType.bypass,
                            replica_groups=[list(range(NC))],
                            ins=[cc_in.ap().opt()],
                            outs=[cc_out.ap().opt()],
                        )
                        # dummy MMs anchored on this step's hT (so Tile can't
                        # hoist them) and with no AG dependency: they execute
                        # during the collective wait and keep HAM at 8/8
                        for w in range(n_warm):
                            nc.tensor.matmul(warm_ps[:, :], hT[:, :],
                                             wT_sb[:, (w % 8) * 512:(w % 8) * 512 + 512],
                                             start=True, stop=True)
                        hT_all = sp.tile([128, NC, B], DT, tag="hT_all")
                        nc.sync.dma_start(
                            out=hT_all[:, :, :],
                            in_=cc_out.ap().rearrange("(k p) b -> p k b", k=NC),
                        )
                        hT_all_prev = hT_all
                        nc.vector.tensor_copy(seqT[:, :, t, :], hT_all[:, :, bsel])
                    else:
                        hT = sp.tile([HS, B], DT, tag="hTsb")
                        nc.scalar.copy(out=hT[:, :], in_=hT_ps[:, :])
                        nc.gpsimd.dma_start(out=cc_in[:, :], in_=hT[:, :])
                        if comm == 'ag':
                            nc.gpsimd.collective_compute(
                                "AllGather",
                                mybir.AluOpType.bypass,
                                replica_groups=[list(range(NC))],
                                ins=[cc_in.ap().opt()],
                                outs=[cc_out.ap().opt()],
                            )
                        else:
                            nc.gpsimd.dma_start(out=cc_out[0:HS, :], in_=cc_in[:, :])
                        hT_all = sp.tile([128, NC, B], DT, tag="hT_all")
                        nc.gpsimd.dma_start(
                            out=hT_all[:, :, :],
                            in_=cc_out.ap().rearrange("(k p) b -> p k b", k=NC),
                        )
                        hT_all_prev = hT_all
                        # stash my batch slice: seqT[:, t, k, b] = hT_all[:, k, 8p+b]
                        nc.vector.tensor_copy(seqT[:, :, t, :], hT_all[:, :, bsel])

            # ================= phase 2: LN + projection =================
            with (
                tc.tile_pool(name="p2", bufs=2) as p2,
                tc.tile_pool(name="ps2", bufs=2, space="PSUM") as pp2,
            ):
                inv_h = 1.0 / H
                for tg_i in range(n_tg):
                    t0 = tg_i * 64
                    tn = min(64, t_steps - t0)
                    ncols = tn * NC * BL

                    # squares
                    sq = p2.tile([128, NC, 64, BL], DT, tag="sq")
                    nc.scalar.activation(sq[:, :, 0:tn, :],
                                         seqT[:, :, t0:t0 + tn, :], AF.Square)

                    # partition sums via ones-matmul, accumulated over k-chunks
                    mu_ps = pp2.tile([1, 512], F32, tag="mu", bufs=1)
                    sqs_ps = pp2.tile([1, 512], F32, tag="sqs", bufs=1)
                    for k in range(NC):
                        nc.tensor.matmul(
                            mu_ps[:, 0:tn * BL], ones128[:, :],
                            seqT[:, k, t0:t0 + tn, :].opt(),
                            start=(k == 0), stop=(k == NC - 1))
                    for k in range(NC):
                        nc.tensor.matmul(
                            sqs_ps[:, 0:tn * BL], ones128[:, :],
                            sq[:, k, 0:tn, :].opt(),
                            start=(k == 0), stop=(k == NC - 1))

                    # stats: mean, var, rstd, -rstd*mean
                    mean = p2.tile([1, 512], F32, tag="mean")
                    nc.vector.tensor_scalar_mul(mean[:, 0:tn * BL],
                                                mu_ps[:, 0:tn * BL], inv_h)
                    exsq = p2.tile([1, 512], F32, tag="exsq")
                    nc.vector.tensor_scalar_mul(exsq[:, 0:tn * BL],
                                                sqs_ps[:, 0:tn * BL], inv_h)
                    msq = p2.tile([1, 512], F32, tag="msq")
                    nc.vector.tensor_mul(msq[:, 0:tn * BL], mean[:, 0:tn * BL],
                                         mean[:, 0:tn * BL])
                    var = p2.tile([1, 512], F32, tag="var")
                    nc.vector.tensor_sub(var[:, 0:tn * BL], exsq[:, 0:tn * BL],
                                         msq[:, 0:tn * BL])
                    std = p2.tile([1, 512], F32, tag="std")
                    nc.scalar.activation(std[:, 0:tn * BL], var[:, 0:tn * BL],
                                         AF.Sqrt, bias=eps_t[0:1, 0:1])
                    rstd = p2.tile([1, 512], F32, tag="rstd")
                    nc.vector.reciprocal(rstd[:, 0:tn * BL], std[:, 0:tn * BL])
                    negrm = p2.tile([1, 512], DT, tag="negrm")
                    nc.vector.scalar_tensor_tensor(
                        negrm[:, 0:tn * BL], rstd[:, 0:tn * BL], -1.0,
                        mean[:, 0:tn * BL],
                        op0=mybir.AluOpType.mult, op1=mybir.AluOpType.mult)

                    n_mg = (tn + 15) // 16
                    for sub in range(n_mg):
                        t0m = t0 + sub * 16
                        tm = min(16, t_steps - t0m)
                        mrows = tm * BL

                        # rstd as per-partition column
                        rcol_ps = pp2.tile([128, 1], F32, tag="rcol", bufs=1)
                        nc.tensor.transpose(
                            rcol_ps[0:mrows, :],
                            rstd[:, sub * 128:sub * 128 + mrows],
                            ident[0:1, 0:1])
                        rcol = p2.tile([128, 1], F32, tag="rcolsb")
                        nc.scalar.copy(out=rcol[0:mrows, :], in_=rcol_ps[0:mrows, :])

                        # projection
                        p_ps = pp2.tile([128, D], F32, tag="P")
                        for k in range(NC):
                            nc.tensor.matmul(
                                p_ps[0:mrows, :],
                                seqT[:, k, t0m:t0m + tm, :].opt(),
                                w1T_sb[:, k * D:(k + 1) * D],
                                start=(k == 0), stop=(k == NC - 1))
                        # rank-1 terms in a separate accumulator
                        r1_ps = pp2.tile([128, D], F32, tag="r1")
                        nc.tensor.matmul(r1_ps[0:mrows, :],
                                         negrm[:, sub * 128:sub * 128 + mrows],
                                         a_sb[:, :], start=True, stop=False)
                        nc.tensor.matmul(r1_ps[0:mrows, :],
                                         ones_row[:, 0:mrows],
                                         c_sb[:, :], start=False, stop=True)

                        scaled = p2.tile([128, D], F32, tag="scaled")
                        nc.scalar.activation(scaled[0:mrows, :], p_ps[0:mrows, :],
                                             AF.Copy, scale=rcol[0:mrows, :])
                        fin = p2.tile([128, D], F32, tag="fin")
                        nc.vector.tensor_add(fin[0:mrows, :], scaled[0:mrows, :],
                                             r1_ps[0:mrows, :])
                        g16 = tg_i * n_mg_per_tg + sub
                        nc.sync.dma_start(
                            out=out[g16 * 128:g16 * 128 + mrows, :],
                            in_=fin[0:mrows, :])

    # Cross-core gates attached AFTER Tile scheduling: Tile's single-core
    # scheduling sim can't satisfy remotely-incremented semaphores.
    for inst, sem, val in deferred_waits:
        inst.wait_op(sem, val, "sem-ge", check=False)
    nc.finalize()
    return nc


def _prep_inputs(x, W_ih, W_hh, b_ih, b_hh, ln_gamma, ln_beta, W_out, b_out,
                 comm='ag', prec='f32r'):
    """Host-side sharding. Returns per-core input maps.

    For comm='rd' the K-chunk slots are XOR-permuted per core: slot j on
    core c holds chunk c^j (matching the remote-dma broadcast pattern)."""
    npdt = np.float16 if prec == 'f16' else np.float32
    Wc = (W_ih + W_hh).astype(np.float32)
    bb = (b_ih + b_hh).astype(np.float32)

    def chunk_major_T(wslice, perm):
        # [512, 1024] -> [128, 8*512]: slot s <- K-chunk perm[s]
        wt = wslice.T.reshape(NC, 128, wslice.shape[0])[perm]
        return np.ascontiguousarray(wt.transpose(1, 0, 2).reshape(128, -1))

    def x_cm(perm):
        return np.ascontiguousarray(
            x.T.reshape(NC, 128, B)[perm].transpose(1, 0, 2).reshape(128, NC * B)
        ).astype(np.float32)

    W1 = (W_out * ln_gamma[None, :]).astype(np.float32)       # [512, 1024]
    a_v = W1.sum(axis=1).astype(np.float32).reshape(1, D)
    c_v = (ln_beta @ W_out.T + b_out).astype(np.float32).reshape(1, D)
    identB = np.eye(B, dtype=np.float32)
    ones128 = np.ones((128, 1), dtype=np.float32)

    in_maps = []
    for c in range(NC):
        perm = [c ^ j for j in range(NC)] if comm == 'rd' else list(range(NC))

        def rows(W):
            return np.concatenate([
                W[1024 + 128 * c:1024 + 128 * c + 128],   # f
                W[0 + 128 * c:0 + 128 * c + 128],          # i
                W[3072 + 128 * c:3072 + 128 * c + 128],    # o
                W[2048 + 128 * c:2048 + 128 * c + 128],    # g
            ], axis=0)

        bias_fiog = np.concatenate([
            bb[1024 + 128 * c:1024 + 128 * c + 128],
            bb[0 + 128 * c:0 + 128 * c + 128],
            bb[3072 + 128 * c:3072 + 128 * c + 128],
            bb[2048 + 128 * c:2048 + 128 * c + 128],
        ]).reshape(1, 512).astype(np.float32)

        in_maps.append({
            "w0T": chunk_major_T(rows(W_ih.astype(np.float32)), perm).astype(npdt),
            "wT": chunk_major_T(rows(Wc), perm).astype(npdt),
            "biasT": bias_fiog.astype(npdt),
            "xT": x_cm(perm).astype(npdt),
            "identB": identB,
            "w1T": chunk_major_T(W1, perm).astype(npdt),
            "avec": a_v.astype(npdt),
            "cvec": c_v.astype(npdt),
            "ones128": ones128.astype(npdt),
            "onesB": np.ones((1, B), dtype=npdt),
            "ones_row": np.ones((1, 128), dtype=npdt),
        })
    return in_maps


def kernel(x, W_ih, W_hh, b_ih, b_hh, ln_gamma, ln_beta, W_out, b_out,
           _t=T_FULL, _trace=False, _comm='ag2', _prec='f16'):
    t_steps = _t
    key = (t_steps, _comm, _prec)
    if key not in _CACHE:
        _CACHE[key] = _build(t_steps, _comm, _prec)
    nc = _CACHE[key]

    in_maps = _prep_inputs(x, W_ih, W_hh, b_ih, b_hh,
                           ln_gamma, ln_beta, W_out, b_out, comm=_comm,
                           prec=_prec)
    res = run_bass_kernel_spmd(nc, in_maps, core_ids=list(range(NC)),
                               trace=_trace)

    out = np.empty((B, t_steps, D), dtype=np.float32)
    for c in range(NC):
        r = res.results[c]["out"][:t_steps * BL].reshape(t_steps, BL, D)
        out[BL * c:BL * c + BL] = r.transpose(1, 0, 2)
    if _trace:
        kernel.last_result = res
    return out

